# revision 34
# baseline (speedup 1.0000x reference)
"""MoE layer (16 experts, top-2) on 8 Trainium2 NeuronCores, expert-parallel.

Strategy (v2):
  - Host computes gating (logits -> top-k -> softmax) and buckets tokens per
    expert. Each core owns 2 experts (slot 0: one of the 8 biggest buckets,
    slot 1: one of the 8 smallest), so the two slots get separate compile-time
    token capacities CA >= CB instead of one global max -- less zero padding.
  - All matmul operands are bf16 (PSUM accumulation stays fp32). bf16 streams
    at full PE rate like fp32r but LDWEIGHTS gets fast-weight-load (2x) so it
    hides completely under the matmuls, and DMA bytes halve.
  - mm1: h^T[f, tok] = silu(W1^T x^T + b1), stationary = W1 [d,f] chunks,
    moving = tokens.  mm2 produces y^T[d, tok] with stationary = W2 [f,d]
    chunks (natural layout) and moving = tokens: no token-chunk padding and
    no on-device gating; the host applies gate weights during the combine.
  - Input DMAs ride the two HWDGE queues (sync/scalar) split so each weight
    tensor streams on one queue with ~1MB chunks; x^T goes first on both.
    A short burst of warm-up matmuls on a zeroed tile keeps the PE busy (and
    the HAM clock-gate warm) while the first real inputs land.
"""

import math

import numpy as np
import ml_dtypes

D_MODEL = 1024
D_FF = 4096
N_EXPERTS = 16
N_CORES = 8
SLOTS = 2  # experts per core
KD = D_MODEL // 128  # 8 contraction chunks for mm1
KF = D_FF // 128  # 32 contraction chunks for mm2
KDO = D_MODEL // 128  # 8 output d-chunks for mm2
FB = 4  # W1 f-chunks per DMA (512 KB)
BF16 = ml_dtypes.bfloat16

_PROG_CACHE: dict[tuple, object] = {}


def _tok_splits(C):
    """Moving-dim tiles (offset, width), each <= 512 (one PSUM bank fp32)."""
    out = []
    o = 0
    while o < C:
        w = min(512, C - o)
        out.append((o, w))
        o += w
    return out


def _build_program(CA, CB):
    import concourse.bass as bass  # noqa: F401
    import concourse.tile as tile
    from concourse import bacc, mybir

    f32 = mybir.dt.float32
    bf = mybir.dt.bfloat16
    Cs = [CA, CB]

    nc = bacc.Bacc("TRN2", target_bir_lowering=False, debug=False, num_devices=N_CORES)

    xt_d = [
        nc.dram_tensor(f"xt{s}", [128, KD, Cs[s]], bf, kind="ExternalInput").ap()
        for s in range(SLOTS)
    ]
    w1_d = nc.dram_tensor("w1", [SLOTS, 128, KF, KD, 128], bf, kind="ExternalInput").ap()
    w2_d = nc.dram_tensor("w2", [SLOTS, 128, KDO, KF, 128], bf, kind="ExternalInput").ap()
    b1_d = nc.dram_tensor("b1", [SLOTS, 128, KF], f32, kind="ExternalInput").ap()
    y_d = [
        nc.dram_tensor(f"y{s}", [D_MODEL, Cs[s]], f32, kind="ExternalOutput").ap()
        for s in range(SLOTS)
    ]

    silu = mybir.ActivationFunctionType.Silu

    with tile.TileContext(nc) as tc:
        with (
            tc.tile_pool(name="warmp", bufs=1) as warmp,
            tc.tile_pool(name="xtp", bufs=1) as xtp,
            tc.tile_pool(name="w1p", bufs=5) as w1p,
            tc.tile_pool(name="w2p", bufs=4) as w2p,
            tc.tile_pool(name="htp", bufs=1) as htp,
            tc.tile_pool(name="smallp", bufs=1) as smallp,
            tc.tile_pool(name="yp", bufs=4) as yp,
            tc.tile_pool(name="psa", bufs=2, space="PSUM") as psa,
            tc.tile_pool(name="psb", bufs=2, space="PSUM") as psb,
            tc.tile_pool(name="qsa", bufs=2, space="PSUM") as qsa,
            tc.tile_pool(name="qsb", bufs=2, space="PSUM") as qsb,
        ):
            # ---- PE warm-up on a zeroed tile while inputs land ----
            wu = warmp.tile([128, 512], bf, name="wu", tag="wu")
            nc.gpsimd.memset(wu[:], 0.0)
            for i in range(12):
                pw = psa.tile([128, 512], f32, name=f"warm{i}", tag="psa")
                nc.tensor.matmul(
                    pw[:], lhsT=wu[:, :128], rhs=wu[:], start=True, stop=True
                )

            # W1 chunk 0 rides at the very front of the sync queue so the
            # first real matmul only waits for it plus x^T's first k-piece
            w1_pre = w1p.tile([128, FB, KD, 128], bf, name="w1t0_0", tag="w1t")
            nc.sync.dma_start(w1_pre[:, :1], w1_d[0, :, 0:1])

            # ---- early input DMAs; slot0's x^T rides the fast HWDGE queues,
            # slot1's (needed much later) + biases go via gpsimd ----
            # slot-0 x^T rides the fast HWDGE queues now; slot-1's is issued
            # later (behind slot-0's W1 on the sync queue) so it does not
            # steal HBM bandwidth from the critical first chunks.
            # slot-0 x^T as one tile per k-chunk, round-robined over the
            # queues: the mm1 f=0 k-loop consumes pieces as they land
            # (separate tiles keep the dependencies per-piece)
            xts = []
            xt0k = []
            engs = [nc.sync, nc.scalar, nc.gpsimd]
            for ki in range(KD):
                t = xtp.tile([128, Cs[0]], bf, name=f"xt0k{ki}", tag=f"xtk{ki}")
                engs[ki % 3].dma_start(t[:], xt_d[0][:, ki])
                xt0k.append(t)
            xts.append(xt0k)
            xt1 = xtp.tile([128, KD, Cs[1]], bf, name="xt1", tag="xt")
            xts.append(xt1)
            b1ts = []
            for s in range(SLOTS):
                b1t = smallp.tile([128, KF], f32, name=f"b1t{s}", tag=f"b1t{s}")
                nc.gpsimd.dma_start(b1t[:], b1_d[s])
                b1ts.append(b1t)

            for s in range(SLOTS):
                C = Cs[s]
                xt = xts[s]
                b1t = b1ts[s]
                splits = _tok_splits(C)
                ht = htp.tile([128, KF, C], bf, name=f"ht{s}", tag=f"ht{s}")

                # ---- mm1: ht[f, c] = silu(W1.T @ XT + b1) ----
                # smaller leading chunks so the first matmuls aren't starved
                fchunks = [1, 1, 2] + [FB] * ((KF - 4) // FB) if s == 0 else [FB] * (
                    KF // FB
                )
                FB2 = 8  # W2 f-chunks per DMA (256 KB)
                w2_pre = {}
                f0 = 0
                for ci, fw in enumerate(fchunks):
                    if s == 0 and ci == 0:
                        w1t = w1_pre
                    else:
                        w1t = w1p.tile(
                            [128, FB, KD, 128], bf, name=f"w1t{s}_{f0}", tag="w1t"
                        )
                        nc.sync.dma_start(w1t[:, :fw], w1_d[s, :, f0 : f0 + fw])
                    if s == 0 and ci == 3:
                        # first two W2 chunks ride sync behind the early W1
                        # chunks instead of competing with x^T on scalar
                        for wf0 in (0, FB2):
                            t = w2p.tile(
                                [128, FB2, 128], bf, name=f"w2t0_0_{wf0}", tag="w2t"
                            )
                            nc.sync.dma_start(t[:], w2_d[s, :, 0, wf0 : wf0 + FB2])
                            w2_pre[(0, wf0)] = t
                    for fb in range(fw):
                        f = f0 + fb
                        pts = []
                        for ti, (o, w) in enumerate(splits):
                            pool, tag = (psa, "psa") if ti == 0 else (psb, "psb")
                            p = pool.tile(
                                [128, w], f32, name=f"p{s}_{f}_{ti}", tag=tag
                            )
                            pts.append(p)
                        for k in range(KD):
                            rk = xt[k] if s == 0 else xt[:, k]
                            for p, (o, w) in zip(pts, splits):
                                nc.tensor.matmul(
                                    p[:, :w],
                                    lhsT=w1t[:, fb, k],
                                    rhs=rk[:, o : o + w],
                                    start=(k == 0),
                                    stop=(k == KD - 1),
                                )
                        for p, (o, w) in zip(pts, splits):
                            nc.scalar.activation(
                                ht[:, f, o : o + w],
                                p[:, :w],
                                silu,
                                bias=b1t[:, f : f + 1],
                            )
                    f0 += fw

                if s == 0:
                    # slot-1 x^T: queued on sync behind slot-0's W1 chunks
                    nc.sync.dma_start(
                        xts[1][:, : KD // 2], xt_d[1][:, : KD // 2]
                    )
                    nc.sync.dma_start(
                        xts[1][:, KD // 2 :], xt_d[1][:, KD // 2 :]
                    )

                # ---- mm2: y^T[d, c] = ht.T-contract @ W2 (W2 natural layout) ----
                for d in range(KDO):
                    qts = []
                    for ti, (o, w) in enumerate(splits):
                        pool, tag = (qsa, "qsa") if ti == 0 else (qsb, "qsb")
                        q = pool.tile([128, w], f32, name=f"q{s}_{d}_{ti}", tag=tag)
                        qts.append(q)
                    for f0 in range(0, KF, FB2):
                        w2t = w2_pre.pop((d, f0), None)
                        if w2t is None:
                            w2t = w2p.tile(
                                [128, FB2, 128], bf, name=f"w2t{s}_{d}_{f0}", tag="w2t"
                            )
                            nc.scalar.dma_start(w2t[:], w2_d[s, :, d, f0 : f0 + FB2])
                        for fb in range(FB2):
                            f = f0 + fb
                            for q, (o, w) in zip(qts, splits):
                                nc.tensor.matmul(
                                    q[:, :w],
                                    lhsT=w2t[:, fb],
                                    rhs=ht[:, f, o : o + w],
                                    start=(f == 0),
                                    stop=(f == KF - 1),
                                )
                    yt = yp.tile([128, C], f32, name=f"yt{s}_{d}", tag="yt")
                    yrow = y_d[s][d * 128 : (d + 1) * 128, :]
                    if s == SLOTS - 1 and d == KDO - 1 and len(splits) == 1:
                        # final output: two half-copies + two parallel HWDGE
                        # DMAs so the kernel-tail drain starts sooner
                        h = (C // 2 + 3) // 4 * 4
                        nc.vector.tensor_copy(yt[:, :h], qts[0][:, :h])
                        nc.scalar.dma_start(yrow[:, :h], yt[:, :h])
                        nc.vector.tensor_copy(yt[:, h:C], qts[0][:, h:C])
                        nc.sync.dma_start(yrow[:, h:], yt[:, h:C])
                    else:
                        for q, (o, w) in zip(qts, splits):
                            nc.vector.tensor_copy(yt[:, o : o + w], q[:, :w])
                        last = s == SLOTS - 1 and d >= KDO - 4
                        yeng = nc.scalar if last else nc.gpsimd
                        yeng.dma_start(yrow, yt[:])

    nc.compile()
    return nc


def _route(x2d, Wg, k):
    logits = x2d.astype(np.float32) @ Wg.astype(np.float32)  # [T, E]
    idx = np.argsort(-logits, axis=1, kind="stable")[:, :k]  # [T, k]
    vals = np.take_along_axis(logits, idx, axis=1)
    e = np.exp(vals - vals.max(axis=1, keepdims=True))
    w = (e / e.sum(axis=1, keepdims=True)).astype(np.float32)
    return idx, w


def kernel(x, W1, b1, W2, b2, Wg, k):
    from concourse.bass_utils import run_bass_kernel_spmd

    x = np.asarray(x, np.float32)
    W1 = np.asarray(W1, np.float32)
    b1 = np.asarray(b1, np.float32)
    W2 = np.asarray(W2, np.float32)
    b2 = np.asarray(b2, np.float32)
    Wg = np.asarray(Wg, np.float32)
    k = int(k)

    B, T, D = x.shape
    x2d = np.ascontiguousarray(x.reshape(-1, D))
    n_tok = x2d.shape[0]

    idx, w = _route(x2d, Wg, k)

    # bucket tokens per expert
    tok_lists, wt_lists = [], []
    for e in range(N_EXPERTS):
        sel = np.nonzero(idx == e)
        tok_lists.append(sel[0].astype(np.int64))
        wt_lists.append(w[sel[0], sel[1]])

    counts = np.array([len(t) for t in tok_lists])
    order = np.argsort(-counts, kind="stable")  # big first
    # slot 0 <- 8 biggest buckets, slot 1 <- 8 smallest
    slot_experts = [order[:N_CORES], order[N_CORES:]]

    def cap(n):
        return max(64, (int(n) + 7) // 8 * 8)

    CA = cap(counts[order[0]])
    CB = cap(counts[order[N_CORES]])
    Cs = [CA, CB]

    key = (CA, CB)
    nc = _PROG_CACHE.get(key)
    if nc is None:
        nc = _build_program(CA, CB)
        _PROG_CACHE[key] = nc

    w1_all = W1.astype(BF16)  # [E, 1024, 4096]
    w2_all = W2.astype(BF16)  # [E, 4096, 1024]
    x_bf = x2d.astype(BF16)

    in_maps = []
    for c in range(N_CORES):
        m = {
            "w1": np.empty((SLOTS, 128, KF, KD, 128), BF16),
            "w2": np.empty((SLOTS, 128, KDO, KF, 128), BF16),
            "b1": np.empty((SLOTS, 128, KF), np.float32),
        }
        for s in range(SLOTS):
            e = int(slot_experts[s][c])
            C = Cs[s]
            toks = tok_lists[e]
            cnt = len(toks)
            xt = np.zeros((128, KD, C), BF16)
            # xt[p, kd, c] = x[token c, kd*128 + p]
            xt[:, :, :cnt] = x_bf[toks].reshape(cnt, KD, 128).transpose(2, 1, 0)
            m[f"xt{s}"] = xt
            # w1[p, f, kd, c] = W1[e, kd*128+p, f*128+c]
            m["w1"][s] = (
                w1_all[e].reshape(KD, 128, KF, 128).transpose(1, 2, 0, 3)
            )
            # w2[p, d, kf, c] = W2[e, kf*128+p, d*128+c]
            m["w2"][s] = (
                w2_all[e].reshape(KF, 128, KDO, 128).transpose(1, 2, 0, 3)
            )
            # b1[p, f] = b1[e, f*128+p]
            m["b1"][s] = b1[e].reshape(KF, 128).T
        in_maps.append(m)

    import os

    trace = bool(os.environ.get("MOE_TRACE"))
    r = run_bass_kernel_spmd(nc, in_maps, list(range(N_CORES)), trace=trace)
    global last_results
    last_results = r
    res = r.results

    out = np.zeros((n_tok, D_MODEL), np.float32)
    for c in range(N_CORES):
        for s in range(SLOTS):
            e = int(slot_experts[s][c])
            toks = tok_lists[e]
            cnt = len(toks)
            if cnt == 0:
                continue
            yT = res[c][f"y{s}"]  # [1024, C], raw expert output
            contrib = yT[:, :cnt].T.astype(np.float32)
            if b2[e].any():
                contrib = contrib + b2[e][None, :]
            out[toks] += wt_lists[e][:, None] * contrib
    return out.reshape(B, T, D_MODEL)
